# revision 21
# baseline (speedup 1.0000x reference)
"""Trainium2 Bass kernel: batched truncated matrix exponential of
skew-symmetrized 256x256 matrices (nn_BatchedExponentialOrthogonalization).

Full input:  w   [512, 256, 256] fp32
Full output: out [512, 256, 256] fp32
  a = (w - w^T)/2 per matrix;  out = I + a + a^2/2! + ... + a^6/6!

Sharding: leading batch dim split across 8 NeuronCores (64 matrices each),
fully data-parallel (SPMD, same NEFF, different slabs).

Math (Horner in a^2; exactly 3 matmuls of 256^3 per matrix, the PE minimum
for a degree-6 polynomial).  The device computes only the a^2..a^6 part:
  A'  = W - W^T  (= 2a); W^T comes from an XBAR DMA-transpose load, so the
        PE runs matmuls only.
  Bp  = A'^T A' = -4 a^2                (mm1)
  Bq  = -Bp/12  = a^2/3                 (ACT psum->SBUF scaled copy)
  T1  = A' + Bq = 2a + a^2/3            (tensor_tensor)
  X1  = A' + 6I                         (tensor_tensor, Pool)
  even pairs:  X2 = 1.5*Bq + X1 (DVE stt);  U = Bq . T1
  odd  pairs:  X2 = Bq + X1 (tt);  U = Bq . T1 + (10/3)*Bq (extra PE
               accumulate patches X2's a^2 deficit through T2 = .15*U + X2)
  T2  = 0.15*U + X2 = 12*(I/2 + a/6 + a^2/24 + a^3/120 + a^4/720)
  V   = Bq . T2 = 4 a^2 * T2final       (mm3)
  out = V/4 -> bf16 = P - I - a         (pure ACT scaled copy)
The host adds back I + a in fp32: res = out + (w - w^T)/2 (+1 on the diag).

Precision: fp16 input/intermediates (matmul operands; psum stays fp32),
bf16 output; the I + a part is exact fp32 on the host.  Rel err ~2e-3
(tolerance 2e-2).

Engines per pair-step (2 matrices): PE 24 matmuls (+4 lambda-accum on odd
pairs); DVE sub/X2/T2 + T1 on odd pairs; Pool X1 + T1 on even pairs
(GPSIMD: tensor_tensor only, no PSUM, no TensorScalarPtr on real HW);
ACT Bq + the V output copy; DMA fp16 in + XBAR W^T + bf16 out.
"""
from contextlib import ExitStack

import numpy as np

import concourse.bass as bass
import concourse.mybir as mybir
import concourse.tile as tile
from concourse.bass_utils import run_bass_kernel_spmd

F32 = mybir.dt.float32
F16 = mybir.dt.float16
BF16 = mybir.dt.bfloat16
N = 256
H = 128
N_CORES = 8
N_MAT_PER_CORE = 64
GROUP = 8
_MAX_WAITS = 1

N_WARM = 20


def _split_multi_waits(nc, max_waits=_MAX_WAITS):
    """This container's walrus accepts at most one sync wait per
    instruction; move excess waits onto no-fuse NOPs inserted immediately
    before, on the same engine (semantically identical - engines execute
    their stream serially)."""
    for f in nc.m.functions:
        for b in f.blocks:
            insts = b.instructions
            if not any(
                i.sync_info and i.sync_info.on_wait
                and len(i.sync_info.on_wait) > max_waits
                for i in insts
            ):
                continue
            new = []
            for inst in insts:
                si = inst.sync_info
                if si and si.on_wait and len(si.on_wait) > max_waits:
                    waits = list(si.on_wait)
                    extra, keep = waits[:-max_waits], waits[-max_waits:]
                    for k in range(0, len(extra), max_waits):
                        nop = mybir.InstNoOp(
                            name=f"I-waitsplit-{nc.next_id()}", ins=[], outs=[])
                        nop.engine = inst.engine
                        nop.bass_nofuse = True
                        nop.sync_info = mybir.SyncInfo(
                            on_wait=extra[k:k + max_waits], on_update=[])
                        new.append(nop)
                    inst.sync_info = mybir.SyncInfo(
                        on_wait=keep, on_update=list(si.on_update or []))
                new.append(inst)
            insts.clear()
            insts.extend(new)


def _build_kernel(n_mat=N_MAT_PER_CORE, group=GROUP, split_waits=True):
    nc = bass.Bass(trn_type="TRN2")
    w = nc.dram_tensor("w", [n_mat, N, N], F16, kind="ExternalInput")
    out = nc.dram_tensor("out", [n_mat, N, N], BF16, kind="ExternalOutput")
    n_groups = n_mat // group
    n_pairs = n_mat // 2
    PPG = group // 2  # pairs per group

    mult = mybir.AluOpType.mult
    add = mybir.AluOpType.add
    sub_op = mybir.AluOpType.subtract

    with ExitStack() as ctx:
        tc = ctx.enter_context(tile.TileContext(nc))
        const_pool = ctx.enter_context(tc.tile_pool(name="const", bufs=1))
        win_pool = ctx.enter_context(tc.tile_pool(name="win", bufs=3))
        wt_pool = ctx.enter_context(tc.tile_pool(name="wt", bufs=3))
        ap_pool = ctx.enter_context(tc.tile_pool(name="apg", bufs=5))
        x1_pool = ctx.enter_context(tc.tile_pool(name="x1g", bufs=3))
        bq_pool = ctx.enter_context(tc.tile_pool(name="bq", bufs=7))
        t1_pool = ctx.enter_context(tc.tile_pool(name="t1", bufs=4))
        x2_pool = ctx.enter_context(tc.tile_pool(name="x2", bufs=5))
        t2_pool = ctx.enter_context(tc.tile_pool(name="t2", bufs=4))
        out_pool = ctx.enter_context(tc.tile_pool(name="outp", bufs=3))
        ps_pool = ctx.enter_context(
            tc.tile_pool(name="pss", bufs=4, space="PSUM"))

        # ---- constants: 6*I in fp16 at pair width ----
        ih_f = const_pool.tile([H, 2 * N], F32, tag="ihf")
        nc.gpsimd.memset(ih_f[:], 0.0)
        for t in range(2):
            nc.gpsimd.affine_select(
                out=ih_f[:, t * N:(t + 1) * N],
                in_=ih_f[:, t * N:(t + 1) * N],
                compare_op=mybir.AluOpType.not_equal,
                fill=6.0, base=t * H, pattern=[[-1, N]],
                channel_multiplier=1)
        i6p = const_pool.tile([H, 2 * 2 * N], F16, tag="i6p")
        for m in range(2):
            nc.vector.tensor_copy(i6p[:, m * 2 * N:(m + 1) * 2 * N], ih_f[:])
        # (10/3)*I [128,128] fp16: lhsT of the odd-pair lambda accumulate
        il_f = const_pool.tile([H, H], F32, tag="ilf")
        nc.gpsimd.memset(il_f[:], 0.0)
        nc.gpsimd.affine_select(
            out=il_f[:], in_=il_f[:], compare_op=mybir.AluOpType.not_equal,
            fill=10.0 / 3.0, base=0, pattern=[[-1, H]], channel_multiplier=1)
        il = const_pool.tile([H, H], F16, tag="il")
        nc.vector.tensor_copy(il[:], il_f[:])
        il2_f = const_pool.tile([H, H], F32, tag="il2f")
        nc.gpsimd.memset(il2_f[:], 0.0)
        nc.gpsimd.affine_select(
            out=il2_f[:], in_=il2_f[:], compare_op=mybir.AluOpType.not_equal,
            fill=20.0 / 3.0, base=0, pattern=[[-1, H]], channel_multiplier=1)
        il2 = const_pool.tile([H, H], F16, tag="il2")
        nc.vector.tensor_copy(il2[:], il2_f[:])

        # ---- PE p-state warm-up + ACT table preload during first DMA ----
        warm = ps_pool.tile([H, 2 * 2 * N], F32, tag="pss")
        for _ in range(N_WARM):
            nc.tensor.matmul(warm[:, :2 * N], i6p[:, :H], i6p[:, :2 * N],
                             start=True, stop=True)
        warm_sb = const_pool.tile([H, 8], F32, tag="warmsb")
        nc.scalar.copy(warm_sb[:], ih_f[:, 0:8])

        def load_group(g):
            win = win_pool.tile([H, group * 2 * N], F16, tag="win")
            hg = group // 2
            for half in range(2):
                m0 = half * hg
                nc.sync.dma_start(
                    win[:, m0 * 2 * N:(m0 + hg) * 2 * N],
                    bass.AP(w, (g * group + m0) * N * N,
                            [[N, H], [N * N, hg], [H * N, 2], [1, N]]))
            wT = wt_pool.tile([H, group * 2 * N], F16, tag="wt")
            nc.sync.dma_start_transpose(
                wT[:, :].rearrange("p (t r) -> p t r", t=2),
                bass.AP(w, g * group * N * N, [[N, group * N], [1, N]]))
            return win, wT

        def sub_pair(win, wT, j):
            ap = ap_pool.tile([H, 2 * 2 * N], F16, tag="apg")
            m0 = 2 * j
            nc.vector.tensor_tensor(
                ap[:, :].rearrange("p (m t c) -> p m t c", m=2, t=2),
                win[:, m0 * 2 * N:(m0 + 2) * 2 * N]
                .rearrange("p (m t c) -> p m t c", m=2, t=2),
                wT[:, :].rearrange("p (t m c) -> p t m c", t=2, m=group)
                .transpose([0, 2, 1, 3])[:, m0:m0 + 2],
                op=sub_op)
            return ap

        def mm_pair(psum, lhs_tile, rhs_tile):
            # psum[h] = M . X per matrix; lhsT blocks (k,i) at h*512+k*256+
            # i*128, rhs row-blocks k at h*512+k*256.
            for h in range(2):
                for i in range(2):
                    for k in range(2):
                        nc.tensor.matmul(
                            psum[:, h * 2 * N + i * N:h * 2 * N + (i + 1) * N],
                            lhs_tile[:, h * 2 * N + k * N + i * H:
                                     h * 2 * N + k * N + (i + 1) * H],
                            rhs_tile[:, h * 2 * N + k * N:
                                     h * 2 * N + (k + 1) * N],
                            start=(k == 0), stop=(k == 1))

        def mm1(ap):
            bp = ps_pool.tile([H, 2 * 2 * N], F32, tag="pss")
            mm_pair(bp, ap, ap)
            return bp

        def bq_op(bp):
            bq = bq_pool.tile([H, 2 * 2 * N], F16, tag="bq")
            nc.scalar.mul(bq[:], bp[:], -1.0 / 12.0)
            return bq

        def t1_op(ap, bq, p):
            t1 = t1_pool.tile([H, 2 * 2 * N], F16, tag="t1")
            eng = nc.vector if p % 4 == 0 else nc.gpsimd
            eng.tensor_tensor(t1[:], ap[:], bq[:], op=add)
            return t1

        def x2_op(bq, ap, lam):
            x2 = x2_pool.tile([H, 2 * 2 * N], F16, tag="x2")
            if lam:
                nc.vector.tensor_tensor(x2[:], bq[:], ap[:], op=add)
            else:
                nc.vector.scalar_tensor_tensor(
                    x2[:], bq[:], 1.5, ap[:], op0=mult, op1=add)
            return x2

        def mm2(bq, t1, lam):
            up = ps_pool.tile([H, 2 * 2 * N], F32, tag="pss")
            for h in range(2):
                for i in range(2):
                    for k in range(2):
                        nc.tensor.matmul(
                            up[:, h * 2 * N + i * N:h * 2 * N + (i + 1) * N],
                            bq[:, h * 2 * N + k * N + i * H:
                               h * 2 * N + k * N + (i + 1) * H],
                            t1[:, h * 2 * N + k * N:h * 2 * N + (k + 1) * N],
                            start=(k == 0), stop=False)
                    if lam:
                        # U += (10/3)*Bq : patches X2's a^2 deficit
                        nc.tensor.matmul(
                            up[:, h * 2 * N + i * N:h * 2 * N + (i + 1) * N],
                            il[:],
                            bq[:, h * 2 * N + i * N:h * 2 * N + (i + 1) * N],
                            start=False, stop=False)
                    # U += 40*I : supplies T2's 6I term (0.15*40)
                    nc.tensor.matmul(
                        up[:, h * 2 * N + i * N:h * 2 * N + (i + 1) * N],
                        il2[:],
                        i6p[:, h * 2 * N + i * N:h * 2 * N + (i + 1) * N],
                        start=False, stop=True)
            return up

        def t2_op(up, x2):
            t2 = t2_pool.tile([H, 2 * 2 * N], F16, tag="t2")
            nc.vector.scalar_tensor_tensor(
                t2[:], up[:], 0.15, x2[:], op0=mult, op1=add)
            return t2

        def mm3(bq, t2):
            vp = ps_pool.tile([H, 2 * 2 * N], F32, tag="pss")
            mm_pair(vp, bq, t2)
            return vp

        def out_copy(vp, wout, j):
            nc.scalar.mul(
                wout[:, (2 * j) * 2 * N:(2 * j + 2) * 2 * N],
                vp[:], 0.25)

        def store_group(g, wout):
            for half in range(2):
                m0 = half * (group // 2)
                nc.sync.dma_start(
                    bass.AP(out, (g * group + m0) * N * N,
                            [[N, H], [N * N, group // 2], [H * N, 2], [1, N]]),
                    wout[:, m0 * 2 * N:(m0 + group // 2) * 2 * N])

        # ---- software-pipelined emission ----
        win_t, wT_t, ap_t, x1_t = {}, {}, {}, {}
        bq_t, t1_t, x2_t, t2_t = {}, {}, {}, {}
        bp_t, up_t, vp_t, wout_t = {}, {}, {}, {}

        win_t[0], wT_t[0] = load_group(0)
        if n_groups > 1:
            win_t[1], wT_t[1] = load_group(1)

        for s in range(n_pairs + 12):
            g = s // PPG
            if s % PPG == 0 and g + 2 < n_groups:
                win_t[g + 2], wT_t[g + 2] = load_group(g + 2)
            p = s  # sub (DVE)
            if 0 <= p < n_pairs:
                gp, j = divmod(p, PPG)
                ap_t[p] = sub_pair(win_t[gp], wT_t[gp], j)
                if j == PPG - 1:
                    win_t.pop(gp, None)
                    wT_t.pop(gp, None)
            p = s - 1  # mm1
            if 0 <= p < n_pairs:
                bp_t[p] = mm1(ap_t[p])
            p = s - 3  # mm2
            if 0 <= p < n_pairs:
                up_t[p] = mm2(bq_t[p], t1_t.pop(p), p % 2 == 1)
            p = s - 5  # mm3
            if 0 <= p < n_pairs:
                vp_t[p] = mm3(bq_t.pop(p), t2_t.pop(p))
            p = s - 1  # Bq (ACT)
            if 0 <= p < n_pairs:
                bq_t[p] = bq_op(bp_t.pop(p))
            p = s - 2  # T1 (Pool on even pairs, DVE on odd), X2 (DVE)
            if 0 <= p < n_pairs:
                t1_t[p] = t1_op(ap_t[p], bq_t[p], p)
                x2_t[p] = x2_op(bq_t[p], ap_t.pop(p), p % 2 == 1)
            p = s - 4  # T2 (DVE)
            if 0 <= p < n_pairs:
                t2_t[p] = t2_op(up_t.pop(p), x2_t.pop(p))
            p = s - 6  # out copy (ACT)
            if 0 <= p < n_pairs:
                gp, j = divmod(p, PPG)
                if j == 0:
                    wout_t[gp] = out_pool.tile(
                        [H, group * 2 * N], BF16, tag="wout", name="wout")
                out_copy(vp_t.pop(p), wout_t[gp], j)
            # group store, two steps after its last out_copy
            p = s - 11
            if p >= 0 and p % PPG == 0 and p // PPG < n_groups:
                store_group(p // PPG, wout_t.pop(p // PPG))

    if split_waits:
        _split_multi_waits(nc)
    return nc


_NC_CACHE = {}


def _postprocess(raw: np.ndarray, w: np.ndarray) -> np.ndarray:
    """res = raw (= P - I - a, bf16) + (w - w^T)/2 + I, in fp32."""
    res = raw.astype(np.float32)
    res += (w - np.swapaxes(w, -1, -2)) * 0.5
    idx = np.arange(N)
    res[:, idx, idx] += 1.0
    return res


def kernel(w: np.ndarray) -> np.ndarray:
    w = np.ascontiguousarray(np.asarray(w, dtype=np.float32))
    n_total = w.shape[0]
    assert w.shape == (n_total, N, N)
    per = n_total // N_CORES
    if per not in _NC_CACHE:
        _NC_CACHE[per] = _build_kernel(n_mat=per)
    nc = _NC_CACHE[per]
    w16 = np.ascontiguousarray(w.astype(np.float16))
    in_maps = [{"w": w16[i * per:(i + 1) * per]} for i in range(N_CORES)]
    res = run_bass_kernel_spmd(nc, in_maps, core_ids=list(range(N_CORES)))
    raw = np.concatenate(
        [np.asarray(r["out"]).astype(np.float32) for r in res.results],
        axis=0)
    return _postprocess(raw, w)
